# revision 4
# baseline (speedup 1.0000x reference)
"""Trainium2 Bass kernel for GroupwiseMMD (8 NeuronCores, SPMD).

Math: mmd = m00 - 2*m01 + m11 with m_ab = w_a^T K w_b / (s_a*s_b),
K = exp(-0.5 * ||z_i - z_j||), z [8192, 256] fp32, w_a = c[:, a] in {0,1}.

Device strategy (per core c of 8):
  - i-block = rows [1024c, 1024c+1024) on the matmul FREE dim.
  - j = all 8192 rows as 64 partition-chunks of 128; per-core data is
    "rolled" so the 8 diagonal chunks land at local positions 0..7
    (identical SPMD program on every core, per-core behavior via data).
  - PSUM sq-tile [128, 1024] accumulated purely on the tensor engine:
      -2*z_j.z_i   (bf16, 2 chunks of K=128)
    + rn_i         (hi/lo bf16 split, K=2 matmul -> fp32-grade precision)
    + 0.25 diag bump (K=128 identity x staircase, diag chunks only;
      keeps sq > 0 at the diagonal so sqrt never sees negatives)
  - ACT pass 1: dist = Sqrt(sq + rn_j) with per-partition fp32 bias rn_j,
    written to a bf16 wave buffer.  ACT pass 2: k = Exp(-0.5 * dist), bf16.
    sqrt/exp live in different ACT table sets, so passes are batched in
    waves of G chunk-tiles to amortize the ~2.7us table loads.
  - Weighted sums on the tensor engine: [w0,w1]^T @ k_tile (M=2 matmul)
    accumulated in PSUM over all 64 j-chunks -> acc [2, 1024].
  - Diagonal of K zeroed via a [128,128] (1-I) mask multiply; the exact
    diagonal contribution (K_ii = 1) is restored on the host in fp64.

Host: gather per-core acc -> a0 = K_off w0, a1 = K_off w1 (full 8192),
combine with exact diag counts (s0, s1, overlap) in float64.
"""

import sys

for _p in ("/opt/trn_rl_repo",):
    if _p not in sys.path:
        sys.path.insert(0, _p)

import numpy as np
import ml_dtypes

N = 8192
D = 256
P = 128
NCORES = 8
IB = N // NCORES          # 1024 i-columns per core
NCH = N // P              # 64 j-chunks
G = 32                    # wave size (chunk-tiles) for ACT table batching
BUMP = 4.0                # diagonal sq bump (>> bf16 matmul noise on sq_ii)

_BF16 = ml_dtypes.bfloat16

_nc_cache = None


def _build_nc():
    import concourse.bass as bass  # noqa: F401
    import concourse.bacc as bacc
    import concourse.mybir as mybir
    import concourse.tile as tile

    f32 = mybir.dt.float32
    bf16 = mybir.dt.bfloat16
    AF = mybir.ActivationFunctionType

    nc = bacc.Bacc()
    zt0 = nc.declare_dram_parameter("zt0", [P, N], bf16, isOutput=False)
    zt1 = nc.declare_dram_parameter("zt1", [P, N], bf16, isOutput=False)
    zi0 = nc.declare_dram_parameter("zi0", [P, IB], bf16, isOutput=False)
    zi1 = nc.declare_dram_parameter("zi1", [P, IB], bf16, isOutput=False)
    rnI = nc.declare_dram_parameter("rnI", [2, IB], bf16, isOutput=False)
    rnP = nc.declare_dram_parameter("rnP", [P, NCH], f32, isOutput=False)
    wL = nc.declare_dram_parameter("wL", [P, 2 * NCH], bf16, isOutput=False)
    ident = nc.declare_dram_parameter("ident", [P, P], bf16, isOutput=False)
    stair = nc.declare_dram_parameter("stair", [P, 512], bf16, isOutput=False)
    maskI = nc.declare_dram_parameter("maskI", [P, P], bf16, isOutput=False)
    acc_out = nc.declare_dram_parameter("acc_out", [2, IB], f32, isOutput=True)

    with tile.TileContext(nc) as tc:
        with (
            tc.tile_pool(name="big", bufs=1) as big,
            tc.tile_pool(name="dist", bufs=G + 2) as distp,
            tc.tile_pool(name="ktile", bufs=3) as kp,
            tc.tile_pool(name="small", bufs=1) as small,
            tc.psum_pool(name="psS", bufs=3) as psS,
            tc.psum_pool(name="psA", bufs=1) as psA,
        ):
            szt0 = big.tile([P, N], bf16)
            szt1 = big.tile([P, N], bf16)
            szi0 = big.tile([P, IB], bf16)
            szi1 = big.tile([P, IB], bf16)
            srnI = big.tile([2, IB], bf16)
            srnP = big.tile([P, NCH], f32)
            swL = big.tile([P, 2 * NCH], bf16)
            sident = big.tile([P, P], bf16)
            sstair = big.tile([P, 512], bf16)
            smaskI = big.tile([P, P], bf16)
            nc.sync.dma_start(out=szt0, in_=zt0[:])
            nc.sync.dma_start(out=szt1, in_=zt1[:])
            nc.sync.dma_start(out=szi0, in_=zi0[:])
            nc.sync.dma_start(out=szi1, in_=zi1[:])
            nc.sync.dma_start(out=srnI, in_=rnI[:])
            nc.sync.dma_start(out=srnP, in_=rnP[:])
            nc.sync.dma_start(out=swL, in_=wL[:])
            nc.sync.dma_start(out=sident, in_=ident[:])
            nc.sync.dma_start(out=sstair, in_=stair[:])
            nc.sync.dma_start(out=smaskI, in_=maskI[:])

            ones2 = small.tile([2, P], bf16)
            nc.vector.memset(ones2, 1.0)

            acc = psA.tile([2, IB], f32)

            for w0 in range(0, NCH, G):
                dist_tiles = []
                # -- sqrt half-wave (ACT stays in the sqrt table set) --
                for jc in range(w0, w0 + G):
                    S = psS.tile([P, IB], f32)
                    for h in range(2):
                        sl = slice(512 * h, 512 * h + 512)
                        jsl = slice(P * jc, P * jc + P)
                        has_bump = jc < 8 and h == jc // 4
                        nc.tensor.matmul(
                            S[:, sl], lhsT=szt0[:, jsl], rhs=szi0[:, sl],
                            start=True, stop=False,
                        )
                        nc.tensor.matmul(
                            S[:, sl], lhsT=szt1[:, jsl], rhs=szi1[:, sl],
                            start=False, stop=False,
                        )
                        nc.tensor.matmul(
                            S[:, sl], lhsT=ones2[:, :], rhs=srnI[:, sl],
                            start=False, stop=not has_bump,
                        )
                        if has_bump:
                            nc.tensor.matmul(
                                S[:, sl], lhsT=sident, rhs=sstair,
                                start=False, stop=True,
                            )
                    dt_ = distp.tile([P, IB], bf16)
                    nc.scalar.activation(
                        out=dt_, in_=S, func=AF.Sqrt,
                        bias=srnP[:, jc : jc + 1], scale=1.0,
                    )
                    dist_tiles.append(dt_)
                # -- exp half-wave (ACT switches to the exp table set) --
                for jc in range(w0, w0 + G):
                    kt = kp.tile([P, IB], bf16)
                    nc.scalar.activation(
                        out=kt, in_=dist_tiles[jc - w0], func=AF.Exp, scale=-0.5,
                    )
                    if jc < 8:
                        dsl = slice(P * jc, P * jc + P)
                        nc.vector.tensor_mul(
                            out=kt[:, dsl], in0=kt[:, dsl], in1=smaskI
                        )
                    for h in range(2):
                        sl = slice(512 * h, 512 * h + 512)
                        nc.tensor.matmul(
                            acc[:, sl],
                            lhsT=swL[:, 2 * jc : 2 * jc + 2],
                            rhs=kt[:, sl],
                            start=(jc == 0),
                            stop=(jc == NCH - 1),
                        )
            accS = small.tile([2, IB], f32)
            nc.vector.tensor_copy(out=accS, in_=acc)
            nc.sync.dma_start(out=acc_out[:], in_=accS)
    nc.compile()
    return nc


def _get_nc():
    global _nc_cache
    if _nc_cache is None:
        _nc_cache = _build_nc()
    return _nc_cache


def _prep_inputs(c, z_sample):
    z = np.asarray(z_sample, dtype=np.float32)
    carr = np.asarray(c, dtype=np.int32)
    rn = (z.astype(np.float64) ** 2).sum(axis=1)  # [N] exact-ish row norms
    rn32 = rn.astype(np.float32)
    zT = np.ascontiguousarray(z.T)                # [D, N]

    zt_bf = zT.astype(_BF16)                      # j-side, unscaled
    w_bf = carr.astype(_BF16)                     # [N, 2]

    # rn hi/lo split so K=2 bf16 matmul reconstructs rn_i to ~2^-16 rel
    rn_hi = rn32.astype(_BF16)
    rn_lo = (rn32 - rn_hi.astype(np.float32)).astype(_BF16)

    identity = np.eye(P, dtype=_BF16)
    maskI = (1.0 - np.eye(P, dtype=np.float32)).astype(_BF16)
    stair = np.zeros((P, 512), dtype=np.float32)
    for r in range(4):
        stair[np.arange(P), 128 * r + np.arange(P)] = BUMP
    stair = stair.astype(_BF16)

    in_maps = []
    for core in range(NCORES):
        i0 = IB * core
        perm = [(jc + 8 * core) % NCH for jc in range(NCH)]
        zt_p0 = np.empty((P, N), dtype=_BF16)
        zt_p1 = np.empty((P, N), dtype=_BF16)
        wLm = np.empty((P, 2 * NCH), dtype=_BF16)
        rnPm = np.empty((P, NCH), dtype=np.float32)
        for jc, g in enumerate(perm):
            zt_p0[:, P * jc : P * jc + P] = zt_bf[:P, P * g : P * g + P]
            zt_p1[:, P * jc : P * jc + P] = zt_bf[P:, P * g : P * g + P]
            wLm[:, 2 * jc : 2 * jc + 2] = w_bf[P * g : P * g + P, :]
            rnPm[:, jc] = rn32[P * g : P * g + P]
        zi = (-2.0 * zT[:, i0 : i0 + IB]).astype(_BF16)
        in_maps.append(
            {
                "zt0": zt_p0,
                "zt1": zt_p1,
                "zi0": np.ascontiguousarray(zi[:P]),
                "zi1": np.ascontiguousarray(zi[P:]),
                "rnI": np.stack([rn_hi[i0 : i0 + IB], rn_lo[i0 : i0 + IB]]),
                "rnP": rnPm,
                "wL": wLm,
                "ident": identity,
                "stair": stair,
                "maskI": maskI,
            }
        )
    return in_maps


def _combine(c, acc_list):
    carr = np.asarray(c, dtype=np.int64)
    w0 = carr[:, 0].astype(np.float64)
    w1 = carr[:, 1].astype(np.float64)
    s0 = w0.sum()
    s1 = w1.sum()
    ov = float((w0 * w1).sum())
    a0 = np.concatenate([a[0].astype(np.float64) for a in acc_list])
    a1 = np.concatenate([a[1].astype(np.float64) for a in acc_list])
    p00 = float(w0 @ a0) + s0
    p01 = float(w1 @ a0) + ov
    p11 = float(w1 @ a1) + s1
    mmd = p00 / (s0 * s0) - 2.0 * p01 / (s0 * s1) + p11 / (s1 * s1)
    return np.float32(mmd)


def run_device(c, z_sample, **spmd_kwargs):
    """Run the Bass kernel; returns (acc_list, BassKernelResults)."""
    from concourse.bass_utils import run_bass_kernel_spmd

    nc = _get_nc()
    in_maps = _prep_inputs(c, z_sample)
    res = run_bass_kernel_spmd(nc, in_maps, list(range(NCORES)), **spmd_kwargs)
    acc_list = [res.results[i]["acc_out"] for i in range(NCORES)]
    return acc_list, res


def kernel(c, z_sample):
    acc_list, _ = run_device(c, z_sample)
    return _combine(c, acc_list)


# revision 16
# speedup vs baseline: 1.4380x; 1.4380x over previous
"""Trainium2 Bass kernel for GroupwiseMMD (8 NeuronCores, SPMD).

Math: mmd = m00 - 2*m01 + m11 with m_ab = w_a^T K w_b / (s_a*s_b),
K = exp(-0.5 * ||z_i - z_j||), z [8192, 256] fp32, w_a = c[:, a] in {0,1}.

Device strategy (per core c of 8):
  - i-block = rows [1024c, 1024c+1024) on the matmul FREE dim.
  - j = all 8192 rows as 64 partition-chunks of 128; per-core data is
    "rolled" so the 8 diagonal chunks land at local positions 0..7
    (identical SPMD program on every core, per-core behavior via data).
  - PSUM sq-tile [128, 1024] accumulated purely on the tensor engine:
      -2*z_j.z_i   (bf16, 2 chunks of K=128)
    + rn_i         (hi/lo bf16 split, K=2 matmul -> fp32-grade precision)
    + 0.25 diag bump (K=128 identity x staircase, diag chunks only;
      keeps sq > 0 at the diagonal so sqrt never sees negatives)
  - ACT pass 1: dist = Sqrt(sq + rn_j) with per-partition fp32 bias rn_j,
    written to a bf16 wave buffer.  ACT pass 2: k = Exp(-0.5 * dist), bf16.
    sqrt/exp live in different ACT table sets, so passes are batched in
    waves of G chunk-tiles to amortize the ~2.7us table loads.
  - Weighted sums on the tensor engine: [w0,w1]^T @ k_tile (M=2 matmul)
    accumulated in PSUM over all 64 j-chunks -> acc [2, 1024].
  - Diagonal of K zeroed via a [128,128] (1-I) mask multiply; the exact
    diagonal contribution (K_ii = 1) is restored on the host in fp64.

Host: gather per-core acc -> a0 = K_off w0, a1 = K_off w1 (full 8192),
combine with exact diag counts (s0, s1, overlap) in float64.
"""

import sys

for _p in ("/opt/trn_rl_repo",):
    if _p not in sys.path:
        sys.path.insert(0, _p)

import numpy as np
import ml_dtypes

N = 8192
D = 256
P = 128
NCORES = 8
IB = N // NCORES          # 1024 i-columns per core
NCH = N // P              # 64 j-chunks
WAVES = [48, 16]          # wave sizes (chunk-tiles) for ACT table batching
EXPC = 16                 # j-chunks per exp instruction ([128, 16384])
BUMP = 4.0                # diagonal sq bump (>> bf16 matmul noise on sq_ii)

_BF16 = ml_dtypes.bfloat16

_nc_cache = None


def _build_nc():
    import concourse.bass as bass  # noqa: F401
    import concourse.bacc as bacc
    import concourse.mybir as mybir
    import concourse.tile as tile

    f32 = mybir.dt.float32
    bf16 = mybir.dt.bfloat16
    AF = mybir.ActivationFunctionType

    nc = bacc.Bacc()
    zt0 = nc.declare_dram_parameter("zt0", [P, N], bf16, isOutput=False)
    zt1 = nc.declare_dram_parameter("zt1", [P, N], bf16, isOutput=False)
    zi0 = nc.declare_dram_parameter("zi0", [P, IB], bf16, isOutput=False)
    zi1 = nc.declare_dram_parameter("zi1", [P, IB], bf16, isOutput=False)
    rnIb = nc.declare_dram_parameter("rnIb", [P, IB], f32, isOutput=False)
    rnP = nc.declare_dram_parameter("rnP", [P, NCH], f32, isOutput=False)
    wL = nc.declare_dram_parameter("wL", [P, 2 * NCH], bf16, isOutput=False)
    ident = nc.declare_dram_parameter("ident", [P, P], bf16, isOutput=False)
    stair = nc.declare_dram_parameter("stair", [P, 512], bf16, isOutput=False)
    maskI = nc.declare_dram_parameter("maskI", [P, P], bf16, isOutput=False)
    acc_out = nc.declare_dram_parameter("acc_out", [2, IB], f32, isOutput=True)

    with tile.TileContext(nc) as tc:
        with (
            tc.tile_pool(name="big", bufs=1) as big,
            tc.tile_pool(name="dist", bufs=1) as distp,
            tc.tile_pool(name="ktile", bufs=2) as kp,
            tc.tile_pool(name="small", bufs=1) as small,
            tc.psum_pool(name="psS", bufs=3) as psS,
            tc.psum_pool(name="psA", bufs=1) as psA,
        ):
            szt0 = big.tile([P, N], bf16)
            szt1 = big.tile([P, N], bf16)
            szi0 = big.tile([P, IB], bf16)
            szi1 = big.tile([P, IB], bf16)
            srnIb = big.tile([P, IB], f32)
            srnP = big.tile([P, NCH], f32)
            swL = big.tile([P, 2 * NCH], bf16)
            sident = big.tile([P, P], bf16)
            sstair = big.tile([P, 512], bf16)
            smaskI = big.tile([P, P], bf16)
            for s in range(8):
                ssl = slice(s * (N // 8), (s + 1) * (N // 8))
                nc.sync.dma_start(out=szt0[:, ssl], in_=zt0[:, ssl])
                nc.gpsimd.dma_start(out=szt1[:, ssl], in_=zt1[:, ssl])
            nc.sync.dma_start(out=szi0, in_=zi0[:])
            nc.sync.dma_start(out=szi1, in_=zi1[:])
            nc.sync.dma_start(out=srnIb, in_=rnIb[:])
            nc.sync.dma_start(out=srnP, in_=rnP[:])
            nc.sync.dma_start(out=swL, in_=wL[:])
            nc.sync.dma_start(out=sident, in_=ident[:])
            nc.sync.dma_start(out=sstair, in_=stair[:])
            nc.sync.dma_start(out=smaskI, in_=maskI[:])

            acc = psA.tile([2, IB], f32)

            w0 = 0
            for wsz in WAVES:
                # one contiguous dist buffer per wave: sqrt slices write into
                # it, wide exp instructions read it; no per-tile recycling so
                # the scheduler cannot slide waves into each other (each
                # slide costs a ~2.7us ACT table re-load)
                dist = distp.tile([P, wsz * IB], bf16)
                # -- sqrt half-wave (ACT stays in the sqrt table set) --
                for jc in range(w0, w0 + wsz):
                    S = psS.tile([P, IB], f32)
                    jsl = slice(P * jc, P * jc + P)
                    bump_h = jc // 4 if jc < 8 else -1
                    # d-outer / h-inner so consecutive matmuls share lhsT
                    for d, (zt, zi) in enumerate(((szt0, szi0), (szt1, szi1))):
                        for h in range(2):
                            sl = slice(512 * h, 512 * h + 512)
                            nc.tensor.matmul(
                                S[:, sl], lhsT=zt[:, jsl], rhs=zi[:, sl],
                                start=(d == 0),
                                stop=(d == 1 and h != bump_h),
                            )
                    if bump_h >= 0:
                        sl = slice(512 * bump_h, 512 * bump_h + 512)
                        nc.tensor.matmul(
                            S[:, sl], lhsT=sident, rhs=sstair,
                            start=False, stop=True,
                        )
                    # rn_i (free-dim broadcast) on the otherwise-idle DVE
                    nc.vector.tensor_add(out=S, in0=S, in1=srnIb)
                    nc.scalar.activation(
                        out=dist[:, (jc - w0) * IB : (jc - w0 + 1) * IB],
                        in_=S, func=AF.Sqrt,
                        bias=srnP[:, jc : jc + 1], scale=1.0,
                    )
                # -- exp half-wave (ACT switches to the exp table set) --
                for jc0 in range(w0, w0 + wsz, EXPC):
                    kt = kp.tile([P, EXPC * IB], bf16)
                    lo = (jc0 - w0) * IB
                    nc.scalar.activation(
                        out=kt, in_=dist[:, lo : lo + EXPC * IB],
                        func=AF.Exp, scale=-0.5,
                    )
                    for jc in range(jc0, jc0 + EXPC):
                        if jc < 8:
                            dsl = slice((jc - jc0) * IB + P * jc,
                                        (jc - jc0) * IB + P * jc + P)
                            nc.vector.tensor_mul(
                                out=kt[:, dsl], in0=kt[:, dsl], in1=smaskI
                            )
                        for h in range(2):
                            sl = slice((jc - jc0) * IB + 512 * h,
                                       (jc - jc0) * IB + 512 * h + 512)
                            nc.tensor.matmul(
                                acc[:, 512 * h : 512 * h + 512],
                                lhsT=swL[:, 2 * jc : 2 * jc + 2],
                                rhs=kt[:, sl],
                                start=(jc == 0),
                                stop=(jc == NCH - 1),
                            )
                w0 += wsz
            accS = small.tile([2, IB], f32)
            nc.vector.tensor_copy(out=accS, in_=acc)
            nc.sync.dma_start(out=acc_out[:], in_=accS)
    nc.compile()
    return nc


def _get_nc():
    global _nc_cache
    if _nc_cache is None:
        _nc_cache = _build_nc()
    return _nc_cache


def _prep_inputs(c, z_sample):
    z = np.asarray(z_sample, dtype=np.float32)
    carr = np.asarray(c, dtype=np.int32)
    rn = (z.astype(np.float64) ** 2).sum(axis=1)  # [N] exact-ish row norms
    rn32 = rn.astype(np.float32)
    zT = np.ascontiguousarray(z.T)                # [D, N]

    zt_bf = zT.astype(_BF16)                      # j-side, unscaled
    w_bf = carr.astype(_BF16)                     # [N, 2]

    identity = np.eye(P, dtype=_BF16)
    maskI = (1.0 - np.eye(P, dtype=np.float32)).astype(_BF16)
    stair = np.zeros((P, 512), dtype=np.float32)
    for r in range(4):
        stair[np.arange(P), 128 * r + np.arange(P)] = BUMP
    stair = stair.astype(_BF16)

    in_maps = []
    for core in range(NCORES):
        i0 = IB * core
        perm = [(jc + 8 * core) % NCH for jc in range(NCH)]
        zt_p0 = np.empty((P, N), dtype=_BF16)
        zt_p1 = np.empty((P, N), dtype=_BF16)
        wLm = np.empty((P, 2 * NCH), dtype=_BF16)
        rnPm = np.empty((P, NCH), dtype=np.float32)
        for jc, g in enumerate(perm):
            zt_p0[:, P * jc : P * jc + P] = zt_bf[:P, P * g : P * g + P]
            zt_p1[:, P * jc : P * jc + P] = zt_bf[P:, P * g : P * g + P]
            wLm[:, 2 * jc : 2 * jc + 2] = w_bf[P * g : P * g + P, :]
            rnPm[:, jc] = rn32[P * g : P * g + P]
        zi = (-2.0 * zT[:, i0 : i0 + IB]).astype(_BF16)
        in_maps.append(
            {
                "zt0": zt_p0,
                "zt1": zt_p1,
                "zi0": np.ascontiguousarray(zi[:P]),
                "zi1": np.ascontiguousarray(zi[P:]),
                "rnIb": np.ascontiguousarray(
                    np.broadcast_to(rn32[i0 : i0 + IB], (P, IB))
                ),
                "rnP": rnPm,
                "wL": wLm,
                "ident": identity,
                "stair": stair,
                "maskI": maskI,
            }
        )
    return in_maps


def _combine(c, acc_list):
    carr = np.asarray(c, dtype=np.int64)
    w0 = carr[:, 0].astype(np.float64)
    w1 = carr[:, 1].astype(np.float64)
    s0 = w0.sum()
    s1 = w1.sum()
    ov = float((w0 * w1).sum())
    a0 = np.concatenate([a[0].astype(np.float64) for a in acc_list])
    a1 = np.concatenate([a[1].astype(np.float64) for a in acc_list])
    p00 = float(w0 @ a0) + s0
    p01 = float(w1 @ a0) + ov
    p11 = float(w1 @ a1) + s1
    mmd = p00 / (s0 * s0) - 2.0 * p01 / (s0 * s1) + p11 / (s1 * s1)
    return np.float32(mmd)


def run_device(c, z_sample, **spmd_kwargs):
    """Run the Bass kernel; returns (acc_list, BassKernelResults)."""
    from concourse.bass_utils import run_bass_kernel_spmd

    nc = _get_nc()
    in_maps = _prep_inputs(c, z_sample)
    res = run_bass_kernel_spmd(nc, in_maps, list(range(NCORES)), **spmd_kwargs)
    acc_list = [res.results[i]["acc_out"] for i in range(NCORES)]
    return acc_list, res


def kernel(c, z_sample):
    acc_list, _ = run_device(c, z_sample)
    return _combine(c, acc_list)


# revision 20
# speedup vs baseline: 1.4711x; 1.0231x over previous
"""Trainium2 Bass kernel for GroupwiseMMD (8 NeuronCores, SPMD).

Math: mmd = m00 - 2*m01 + m11 with m_ab = w_a^T K w_b / (s_a*s_b),
K = exp(-0.5 * ||z_i - z_j||), z [8192, 256] fp32, w_a = c[:, a] in {0,1}.

Device strategy (per core c of 8):
  - i-block = rows [1024c, 1024c+1024) on the matmul FREE dim.
  - j = all 8192 rows as 64 partition-chunks of 128; per-core data is
    "rolled" so the 8 diagonal chunks land at local positions 0..7
    (identical SPMD program on every core, per-core behavior via data).
  - PSUM sq-tile [128, 1024] accumulated purely on the tensor engine:
      -2*z_j.z_i   (bf16, 2 chunks of K=128)
    + rn_i         (hi/lo bf16 split, K=2 matmul -> fp32-grade precision)
    + 0.25 diag bump (K=128 identity x staircase, diag chunks only;
      keeps sq > 0 at the diagonal so sqrt never sees negatives)
  - ACT pass 1: dist = Sqrt(sq + rn_j) with per-partition fp32 bias rn_j,
    written to a bf16 wave buffer.  ACT pass 2: k = Exp(-0.5 * dist), bf16.
    sqrt/exp live in different ACT table sets, so passes are batched in
    waves of G chunk-tiles to amortize the ~2.7us table loads.
  - Weighted sums on the tensor engine: [w0,w1]^T @ k_tile (M=2 matmul)
    accumulated in PSUM over all 64 j-chunks -> acc [2, 1024].
  - Diagonal of K zeroed via a [128,128] (1-I) mask multiply; the exact
    diagonal contribution (K_ii = 1) is restored on the host in fp64.

Host: gather per-core acc -> a0 = K_off w0, a1 = K_off w1 (full 8192),
combine with exact diag counts (s0, s1, overlap) in float64.
"""

import sys

for _p in ("/opt/trn_rl_repo",):
    if _p not in sys.path:
        sys.path.insert(0, _p)

import numpy as np
import ml_dtypes

N = 8192
D = 256
P = 128
NCORES = 8
IB = N // NCORES          # 1024 i-columns per core
NCH = N // P              # 64 j-chunks
WAVES = [32, 32]          # wave sizes (chunk-tiles) for ACT table batching
EXPC = 16                 # j-chunks per exp instruction ([128, 16384])
BUMP = 4.0                # diagonal sq bump (>> bf16 matmul noise on sq_ii)

_BF16 = ml_dtypes.bfloat16

_nc_cache = None


def _build_nc():
    import concourse.bass as bass  # noqa: F401
    import concourse.bacc as bacc
    import concourse.mybir as mybir
    import concourse.tile as tile

    f32 = mybir.dt.float32
    bf16 = mybir.dt.bfloat16
    AF = mybir.ActivationFunctionType

    nc = bacc.Bacc()
    zt0 = nc.declare_dram_parameter("zt0", [P, N], bf16, isOutput=False)
    zt1 = nc.declare_dram_parameter("zt1", [P, N], bf16, isOutput=False)
    zi0 = nc.declare_dram_parameter("zi0", [P, IB], bf16, isOutput=False)
    zi1 = nc.declare_dram_parameter("zi1", [P, IB], bf16, isOutput=False)
    rnIb = nc.declare_dram_parameter("rnIb", [P, IB], f32, isOutput=False)
    rnP = nc.declare_dram_parameter("rnP", [P, NCH], f32, isOutput=False)
    wL = nc.declare_dram_parameter("wL", [P, 2 * NCH], bf16, isOutput=False)
    ident = nc.declare_dram_parameter("ident", [P, P], bf16, isOutput=False)
    stair = nc.declare_dram_parameter("stair", [P, 512], bf16, isOutput=False)
    maskI = nc.declare_dram_parameter("maskI", [P, P], bf16, isOutput=False)
    acc_out = nc.declare_dram_parameter("acc_out", [2, IB], f32, isOutput=True)

    with tile.TileContext(nc) as tc:
        with (
            tc.tile_pool(name="big", bufs=1) as big,
            tc.tile_pool(name="dist", bufs=1) as distp,
            tc.tile_pool(name="ktile", bufs=3) as kp,
            tc.tile_pool(name="small", bufs=1) as small,
            tc.psum_pool(name="psS", bufs=3) as psS,
            tc.psum_pool(name="psA", bufs=1) as psA,
        ):
            szt0 = big.tile([P, N], bf16)
            szt1 = big.tile([P, N], bf16)
            szi0 = big.tile([P, IB], bf16)
            szi1 = big.tile([P, IB], bf16)
            srnIb = big.tile([P, IB], f32)
            srnP = big.tile([P, NCH], f32)
            swL = big.tile([P, 2 * NCH], bf16)
            sident = big.tile([P, P], bf16)
            sstair = big.tile([P, 512], bf16)
            smaskI = big.tile([P, P], bf16)
            # small tensors first — the first tiles need them immediately
            nc.sync.dma_start(out=szi0, in_=zi0[:])
            nc.sync.dma_start(out=szi1, in_=zi1[:])
            nc.sync.dma_start(out=srnIb, in_=rnIb[:])
            nc.sync.dma_start(out=srnP, in_=rnP[:])
            nc.sync.dma_start(out=swL, in_=wL[:])
            nc.sync.dma_start(out=sident, in_=ident[:])
            nc.sync.dma_start(out=sstair, in_=stair[:])
            nc.sync.dma_start(out=smaskI, in_=maskI[:])
            for s in range(8):
                ssl = slice(s * (N // 8), (s + 1) * (N // 8))
                nc.sync.dma_start(out=szt0[:, ssl], in_=zt0[:, ssl])
                nc.gpsimd.dma_start(out=szt1[:, ssl], in_=zt1[:, ssl])

            acc = psA.tile([2, IB], f32)

            w0 = 0
            for wsz in WAVES:
                # one contiguous dist buffer per wave: sqrt slices write into
                # it, wide exp instructions read it; no per-tile recycling so
                # the scheduler cannot slide waves into each other (each
                # slide costs a ~2.7us ACT table re-load)
                dist = distp.tile([P, wsz * IB], bf16)
                # -- sqrt half-wave (ACT stays in the sqrt table set) --
                for jc in range(w0, w0 + wsz):
                    S = psS.tile([P, IB], f32)
                    jsl = slice(P * jc, P * jc + P)
                    bump_h = jc // 4 if jc < 8 else -1
                    # d-outer / h-inner so consecutive matmuls share lhsT
                    for d, (zt, zi) in enumerate(((szt0, szi0), (szt1, szi1))):
                        for h in range(2):
                            sl = slice(512 * h, 512 * h + 512)
                            nc.tensor.matmul(
                                S[:, sl], lhsT=zt[:, jsl], rhs=zi[:, sl],
                                start=(d == 0),
                                stop=(d == 1 and h != bump_h),
                            )
                    if bump_h >= 0:
                        sl = slice(512 * bump_h, 512 * bump_h + 512)
                        nc.tensor.matmul(
                            S[:, sl], lhsT=sident, rhs=sstair,
                            start=False, stop=True,
                        )
                    # rn_i (free-dim broadcast) on the otherwise-idle DVE
                    nc.vector.tensor_add(out=S, in0=S, in1=srnIb)
                    nc.scalar.activation(
                        out=dist[:, (jc - w0) * IB : (jc - w0 + 1) * IB],
                        in_=S, func=AF.Sqrt,
                        bias=srnP[:, jc : jc + 1], scale=1.0,
                    )
                # -- exp half-wave (ACT switches to the exp table set) --
                for jc0 in range(w0, w0 + wsz, EXPC):
                    kt = kp.tile([P, EXPC * IB], bf16)
                    lo = (jc0 - w0) * IB
                    nc.scalar.activation(
                        out=kt, in_=dist[:, lo : lo + EXPC * IB],
                        func=AF.Exp, scale=-0.5,
                    )
                    for jc in range(jc0, jc0 + EXPC):
                        if jc < 8:
                            dsl = slice((jc - jc0) * IB + P * jc,
                                        (jc - jc0) * IB + P * jc + P)
                            nc.vector.tensor_mul(
                                out=kt[:, dsl], in0=kt[:, dsl], in1=smaskI
                            )
                        for h in range(2):
                            sl = slice((jc - jc0) * IB + 512 * h,
                                       (jc - jc0) * IB + 512 * h + 512)
                            nc.tensor.matmul(
                                acc[:, 512 * h : 512 * h + 512],
                                lhsT=swL[:, 2 * jc : 2 * jc + 2],
                                rhs=kt[:, sl],
                                start=(jc == 0),
                                stop=(jc == NCH - 1),
                            )
                w0 += wsz
            accS = small.tile([2, IB], f32)
            nc.vector.tensor_copy(out=accS, in_=acc)
            nc.sync.dma_start(out=acc_out[:], in_=accS)
    nc.compile()
    return nc


def _get_nc():
    global _nc_cache
    if _nc_cache is None:
        _nc_cache = _build_nc()
    return _nc_cache


def _prep_inputs(c, z_sample):
    z = np.asarray(z_sample, dtype=np.float32)
    carr = np.asarray(c, dtype=np.int32)
    rn = (z.astype(np.float64) ** 2).sum(axis=1)  # [N] exact-ish row norms
    rn32 = rn.astype(np.float32)
    zT = np.ascontiguousarray(z.T)                # [D, N]

    zt_bf = zT.astype(_BF16)                      # j-side, unscaled
    w_bf = carr.astype(_BF16)                     # [N, 2]

    identity = np.eye(P, dtype=_BF16)
    maskI = (1.0 - np.eye(P, dtype=np.float32)).astype(_BF16)
    stair = np.zeros((P, 512), dtype=np.float32)
    for r in range(4):
        stair[np.arange(P), 128 * r + np.arange(P)] = BUMP
    stair = stair.astype(_BF16)

    in_maps = []
    for core in range(NCORES):
        i0 = IB * core
        perm = [(jc + 8 * core) % NCH for jc in range(NCH)]
        zt_p0 = np.empty((P, N), dtype=_BF16)
        zt_p1 = np.empty((P, N), dtype=_BF16)
        wLm = np.empty((P, 2 * NCH), dtype=_BF16)
        rnPm = np.empty((P, NCH), dtype=np.float32)
        for jc, g in enumerate(perm):
            zt_p0[:, P * jc : P * jc + P] = zt_bf[:P, P * g : P * g + P]
            zt_p1[:, P * jc : P * jc + P] = zt_bf[P:, P * g : P * g + P]
            wLm[:, 2 * jc : 2 * jc + 2] = w_bf[P * g : P * g + P, :]
            rnPm[:, jc] = rn32[P * g : P * g + P]
        zi = (-2.0 * zT[:, i0 : i0 + IB]).astype(_BF16)
        in_maps.append(
            {
                "zt0": zt_p0,
                "zt1": zt_p1,
                "zi0": np.ascontiguousarray(zi[:P]),
                "zi1": np.ascontiguousarray(zi[P:]),
                "rnIb": np.ascontiguousarray(
                    np.broadcast_to(rn32[i0 : i0 + IB], (P, IB))
                ),
                "rnP": rnPm,
                "wL": wLm,
                "ident": identity,
                "stair": stair,
                "maskI": maskI,
            }
        )
    return in_maps


def _combine(c, acc_list):
    carr = np.asarray(c, dtype=np.int64)
    w0 = carr[:, 0].astype(np.float64)
    w1 = carr[:, 1].astype(np.float64)
    s0 = w0.sum()
    s1 = w1.sum()
    ov = float((w0 * w1).sum())
    a0 = np.concatenate([a[0].astype(np.float64) for a in acc_list])
    a1 = np.concatenate([a[1].astype(np.float64) for a in acc_list])
    p00 = float(w0 @ a0) + s0
    p01 = float(w1 @ a0) + ov
    p11 = float(w1 @ a1) + s1
    mmd = p00 / (s0 * s0) - 2.0 * p01 / (s0 * s1) + p11 / (s1 * s1)
    return np.float32(mmd)


def run_device(c, z_sample, **spmd_kwargs):
    """Run the Bass kernel; returns (acc_list, BassKernelResults)."""
    from concourse.bass_utils import run_bass_kernel_spmd

    nc = _get_nc()
    in_maps = _prep_inputs(c, z_sample)
    res = run_bass_kernel_spmd(nc, in_maps, list(range(NCORES)), **spmd_kwargs)
    acc_list = [res.results[i]["acc_out"] for i in range(NCORES)]
    return acc_list, res


def kernel(c, z_sample):
    acc_list, _ = run_device(c, z_sample)
    return _combine(c, acc_list)
